# revision 30
# baseline (speedup 1.0000x reference)
"""Trainium2 Bass kernel for nn_AdvancedRegressionModel (20-qubit circuit regression).

Math: the reference circuit collapses to out_b = sum_j g_j |(A psi_b)_j|^2 + b0
where A = kron of 20 fused 2x2 gates (RY_k @ RX_k per wire) and g absorbs the
CNOT chain (a basis permutation), the <Z_i> measurements, and the linear head
via prefix-sign weights.

Wire blocks: q = wires 0-6 (a:0-5 + m1:6), m2 = wires 7-12, z = wires 13-19.
Per batch row (4 rows/core, batch-sharded over 8 cores):
  load  [q:128, (m2:64, z:128)] fp16 in 16 contiguous chunk DMAs      (DMA)
  P0    contract q: data-stationary matmuls -> PSUM [z, (a | rh,t,h)] (PE)
  drain y [z, (rh, t, k=2*m2+h)] fp16 -- G1's columns are host-permuted
        so P2's stationary picks land CONTIGUOUS in y                 (DVE)
  P2    contract z: stationary = y[:, rh, t, :] (contiguous fp16 ->
        fast weight load), moving G2a/G2b                             (PE)
  drain l2 [k, (rh', t, z')] fp16                                     (DVE)
  P3    contract (m2,h): G3 = kron(Um2, I2) gate-stationary           (PE)
  SQ    ACT square with x512 scale -> fp16 (range-safe)               (ACT)
  P4    reduce with 16-col sign-weight stationary R -> F[10, 8192]    (PE)
  host  finish: out_r = sum_kn F[k,n] * w_k(n) / 512^2 + b

fp16 keeps 10 mantissa bits (same as f32r) -> rel err ~1.4e-3, while halving
DMA/SBUF/LDWEIGHTS cost and freeing matmuls from the f32r self-loading
single-wait constraint (no PE "touch" funnels needed).
"""

import numpy as np
from contextlib import ExitStack

import concourse.mybir as mybir
from concourse import bacc, bass, tile
from concourse.bass_utils import run_bass_kernel_spmd

NW = 20
DIM = 2 ** NW
BATCH = 32
NCORES = 8
RPC = BATCH // NCORES  # rows per core

F32 = mybir.dt.float32
F16 = mybir.dt.float16
SQ_SCALE = 512.0


# ----------------------------------------------------------------- host math
def _gates(params):
    A = []
    for k in range(NW):
        c, s = np.cos(params[k] / 2), np.sin(params[k] / 2)
        RX = np.array([[c, -1j * s], [-1j * s, c]], dtype=np.complex128)
        c2, s2 = np.cos(params[k + NW] / 2), np.sin(params[k + NW] / 2)
        RY = np.array([[c2, -s2], [s2, c2]], dtype=np.complex128)
        A.append(RY @ RX)
    return A


def _kron_list(lst):
    out = lst[0]
    for x in lst[1:]:
        out = np.kron(out, x)
    return out


def _prefix_weights(Wv, wire_lo, wire_hi):
    n = wire_hi - wire_lo + 1
    v = np.arange(2 ** n)
    signs = np.stack([1 - 2 * ((v >> (n - 1 - k)) & 1) for k in range(n)])
    pref = np.cumprod(signs, axis=0)
    g = sum(Wv[wire_lo + i] * pref[i] for i in range(n))
    return g.astype(np.float64), pref[n - 1].astype(np.float64)


def _host_precompute(params, W):
    params = np.asarray(params, dtype=np.float64)
    Wv = np.asarray(W, dtype=np.float64).reshape(-1)
    A = _gates(params)
    Gq = np.kron(_kron_list(A[0:6]), A[6])   # 128x128 on q = (a, m1)
    Uz = _kron_list(A[13:20])                # 128x128 on z
    Um2 = _kron_list(A[7:13])                # 64x64
    G3 = np.kron(Um2, np.eye(2))             # 128x128 on (m2, h)

    # G1 [q:128, nu:256], nu = rh*128 + t*2 + h with q' = h*64 + t
    G1 = np.zeros((128, 256))
    qp = np.arange(128)
    h, t = qp >> 6, qp & 63
    nu_re = t * 2 + h
    G1[:, nu_re] = Gq.real.T
    G1[:, 128 + nu_re] = Gq.imag.T

    G2a = np.concatenate([Uz.real.T, Uz.imag.T], axis=1)    # [z, 256]
    G2b = np.concatenate([-Uz.imag.T, Uz.real.T], axis=1)
    # SQ_SCALE folded into G3S so squares need no ACT scale param
    G3S = SQ_SCALE * np.concatenate(
        [G3.real.T, G3.imag.T, -G3.imag.T], axis=1)

    ga, sA = _prefix_weights(Wv, 0, 5)    # [64]
    gm, sM = _prefix_weights(Wv, 6, 12)   # [128]
    gz, _ = _prefix_weights(Wv, 13, 19)   # [128]

    # R [p3:128, 16], p3 = m2'*2 + h
    p = np.arange(128)
    m2p, hh = p >> 1, p & 1
    cols = [(hh == 0).astype(float), (hh == 1).astype(float)]
    for h0 in (0, 1):
        for m10 in (0, 1):
            cols.append((hh == h0) * gm[m10 * 64 + m2p])
    for h0 in (0, 1):
        for m10 in (0, 1):
            cols.append((hh == h0) * sM[m10 * 64 + m2p])
    R = np.stack(cols, axis=1)
    R = np.concatenate([R, np.zeros((128, 6))], axis=1)

    # free weights w[k, n], n = t*128 + z'; a' = h*32 + (t>>1), m1' = t&1
    n = np.arange(8192)
    t_, zp = n >> 7, n & 127
    m1p = t_ & 1
    w = np.zeros((10, 8192))
    w[0] = ga[0 * 32 + (t_ >> 1)]
    w[1] = ga[1 * 32 + (t_ >> 1)]
    k = 2
    for h0 in (0, 1):
        for m10 in (0, 1):
            w[k] = (m1p == m10) * sA[h0 * 32 + (t_ >> 1)]
            k += 1
    for h0 in (0, 1):
        for m10 in (0, 1):
            w[k] = (m1p == m10) * sA[h0 * 32 + (t_ >> 1)] * gz[zp]
            k += 1
    w /= SQ_SCALE ** 2

    consts = {
        "G1": np.ascontiguousarray(G1, dtype=np.float16),
        "G2a": np.ascontiguousarray(G2a, dtype=np.float16),
        "G2b": np.ascontiguousarray(G2b, dtype=np.float16),
        "G3S": np.ascontiguousarray(G3S, dtype=np.float16),
        "R": np.ascontiguousarray(R, dtype=np.float16),
    }
    return consts, w


def _host_finish(F, w, b):
    # F: [B, 10, 8192]
    out = np.einsum("bkn,kn->b", F.astype(np.float64), w)
    return out + float(np.asarray(b).reshape(-1)[0])


# ------------------------------------------------------------- bass program
def build_bass():
    nc = bacc.Bacc("TRN2", target_bir_lowering=False)
    st = nc.declare_dram_parameter("state", [RPC, DIM], F16, isOutput=False)
    g1 = nc.declare_dram_parameter("G1", [128, 256], F16, isOutput=False)
    g2a = nc.declare_dram_parameter("G2a", [128, 256], F16, isOutput=False)
    g2b = nc.declare_dram_parameter("G2b", [128, 256], F16, isOutput=False)
    g3s = nc.declare_dram_parameter("G3S", [128, 384], F16, isOutput=False)
    rw = nc.declare_dram_parameter("R", [128, 16], F16, isOutput=False)
    fout = nc.declare_dram_parameter("F", [RPC, 10, 8192], F32, isOutput=True)

    with ExitStack() as ctx:
        tc = ctx.enter_context(tile.TileContext(nc))
        const_pool = ctx.enter_context(tc.tile_pool(name="const", bufs=1))
        x0_pool = ctx.enter_context(tc.tile_pool(name="x0", bufs=2))
        y_pool = ctx.enter_context(tc.tile_pool(name="y", bufs=3))
        l2_pool = ctx.enter_context(tc.tile_pool(name="l2", bufs=4))
        sq_pool = ctx.enter_context(tc.tile_pool(name="sq", bufs=4))
        f_pool = ctx.enter_context(tc.tile_pool(name="f", bufs=4))
        psA_pool = ctx.enter_context(
            tc.tile_pool(name="psA", bufs=3, space=bass.MemorySpace.PSUM))
        psB_pool = ctx.enter_context(
            tc.tile_pool(name="psB", bufs=2, space=bass.MemorySpace.PSUM))

        G1 = const_pool.tile([128, 256], F16)
        G2a = const_pool.tile([128, 256], F16)
        G2b = const_pool.tile([128, 256], F16)
        G3S = const_pool.tile([128, 384], F16)
        R = const_pool.tile([128, 16], F16)

        def load_row(r, engines=None, split_first=0):
            # DMA dispatch serializes ~0.6us/instr per dispatching engine;
            # spreading across engines cuts time-to-first-chunk.  The first
            # `split_first` chunks load as four 128-col tiles so the first
            # P0 matmuls only wait on a 32KB transfer.
            if engines is None:
                engines = [nc.gpsimd]
            srcv = st[r].rearrange("(q f) -> q f", q=128, f=8192)
            x0s = []
            for c in range(16):
                eng = engines[c % len(engines)]
                if c < split_first:
                    parts = []
                    for j in range(4):
                        xq = x0_pool.tile([128, 128], F16, tag=f"x0{c}_{j}")
                        lo = 512 * c + 128 * j
                        eng.dma_start(xq[:], srcv[:, lo:lo + 128])
                        parts.append(xq)
                    x0s.append(parts)
                else:
                    xt = x0_pool.tile([128, 512], F16, tag=f"x0{c}")
                    eng.dma_start(xt[:], srcv[:, 512 * c:512 * c + 512])
                    x0s.append(xt)
            return x0s

        # G1 first (needed immediately), then row-0 chunks claim the DMA
        # queues, then the consts needed only from the P2 phase onward.
        nc.sync.dma_start(G1[:], g1[:])
        preload = {0: load_row(0, engines=[nc.sync, nc.gpsimd, nc.scalar])}
        nc.sync.dma_start(G2a[:], g2a[:])
        nc.sync.dma_start(G2b[:], g2b[:])
        nc.sync.dma_start(G3S[:], g3s[:])
        nc.sync.dma_start(R[:], rw[:])

        def make_y(r, x0s):
            """Emit P0 group g for row r; returns per-group closures."""
            y = y_pool.tile([128, 16384], F16, tag="y")
            y_v = y[:].rearrange("p (rh t k) -> p rh t k", rh=2, t=64, k=128)
            y_rt = y[:].rearrange("p (rt k) -> p rt k", rt=128, k=128)

            def p0_group(g):
                ps = psA_pool.tile([128, 1024], F32, tag="ps")
                for q in range(4):
                    xt = x0s[g]
                    xap = (xt[q][:] if isinstance(xt, list)
                           else xt[:, 128 * q:128 * q + 128])
                    nc.tensor.matmul(
                        ps[:, 256 * q:256 * q + 256], xap, G1[:],
                        start=True, stop=True)
                # drain: ps col = a*256 + rt*2 + h -> y col = rt*128+8g+2a+h
                pv = ps[:].rearrange("p (a rt h) -> p a rt h",
                                     a=4, rt=128, h=2)
                dst = y_rt[:, :, 8 * g:8 * g + 8].rearrange(
                    "p rt (a h) -> p a rt h", a=4, h=2)
                if g % 2 == 0:
                    nc.vector.tensor_copy(dst, pv)
                else:
                    nc.scalar.copy(dst, pv)

            return y_v, p0_group

        def stage_p2(r, y_v, g):
            ps2 = psA_pool.tile([128, 1024], F32, tag="ps")
            for q in range(4):
                t = 4 * g + q
                nc.tensor.matmul(ps2[:, 256 * q:256 * q + 256],
                                 y_v[:, 0, t, :], G2a[:],
                                 start=True, stop=False)
                nc.tensor.matmul(ps2[:, 256 * q:256 * q + 256],
                                 y_v[:, 1, t, :], G2b[:],
                                 start=False, stop=True)
            # drain: ps2 cols = (t:4, rh':2, z':128) -> l2 (rh', t, z')
            l2 = l2_pool.tile([128, 1024], F16, tag="l2")
            pv2 = ps2[:].rearrange("p (t rh z) -> p rh t z",
                                   t=4, rh=2, z=128)
            l2_v = l2[:].rearrange("p (rh t z) -> p rh t z",
                                   rh=2, t=4, z=128)
            nc.vector.tensor_copy(l2_v[:, 0], pv2[:, 0])
            nc.vector.tensor_copy(l2_v[:, 1], pv2[:, 1])
            return l2

        def stage_p3(l2):
            psR = psB_pool.tile([128, 512], F32, tag="psB")
            psI = psB_pool.tile([128, 512], F32, tag="psB")
            nc.tensor.matmul(psR[:], G3S[:, 0:128], l2[:, 0:512],
                             start=True, stop=False)
            nc.tensor.matmul(psI[:], G3S[:, 0:128],
                             l2[:, 512:1024], start=True, stop=False)
            nc.tensor.matmul(psR[:], G3S[:, 256:384],
                             l2[:, 512:1024], start=False, stop=True)
            nc.tensor.matmul(psI[:], G3S[:, 128:256],
                             l2[:, 0:512], start=False, stop=True)
            sq = sq_pool.tile([128, 1024], F16, tag="sq")
            nc.scalar.square(sq[:, 0:512], psR[:])
            nc.scalar.square(sq[:, 512:1024], psI[:])
            return sq

        def stage_p4(r, g, sq):
            ps4 = psA_pool.tile([16, 512], F32, tag="ps")
            nc.tensor.matmul(ps4[:], R[:], sq[:, 0:512],
                             start=True, stop=False)
            nc.tensor.matmul(ps4[:], R[:], sq[:, 512:1024],
                             start=False, stop=True)
            fsb = f_pool.tile([10, 512], F32, tag="fsb")
            if g % 2 == 0:
                nc.scalar.copy(fsb[:], ps4[0:10, :])
            else:
                nc.vector.tensor_copy(fsb[:], ps4[0:10, :])
            nc.sync.dma_start(fout[r][:, 512 * g:512 * g + 512], fsb[:])

        # Software-pipelined emission: P3 trails P2 by one group and P4 by
        # two, so every PE wait (l2 drained, squares done) is already
        # satisfied when PE reaches the consuming matmul.  P0 of row r+1
        # interleaves at group granularity; the shared psA pool rotation
        # cycles (ps2, ps4, ps0) kinds.
        y_v, p0g = make_y(0, preload[0])
        for g in range(16):
            p0g(g)
        for r in range(RPC):
            if r + 1 < RPC:
                x0s = load_row(r + 1)
                ny_v, np0g = make_y(r + 1, x0s)
            l2s, sqs = {}, {}
            for g in range(19):
                if g < 16:
                    l2s[g] = stage_p2(r, y_v, g)
                if 1 <= g <= 16:
                    sqs[g - 1] = stage_p3(l2s.pop(g - 1))
                if g >= 3:
                    stage_p4(r, g - 3, sqs.pop(g - 3))
                if g < 16 and r + 1 < RPC:
                    np0g(g)
            if r + 1 < RPC:
                y_v = ny_v
    nc.compile()
    return nc


# ------------------------------------------------------------------ wrapper
_CACHE = {}


def kernel(state, params, W, b):
    state = np.ascontiguousarray(np.asarray(state), dtype=np.float16)
    consts, w = _host_precompute(np.asarray(params), np.asarray(W))

    if "nc" not in _CACHE:
        _CACHE["nc"] = build_bass()
    nc = _CACHE["nc"]

    in_maps = []
    for c in range(NCORES):
        m = {"state": state[RPC * c:RPC * (c + 1)]}
        m.update(consts)
        in_maps.append(m)
    res = run_bass_kernel_spmd(nc, in_maps, list(range(NCORES)))
    F = np.concatenate([res.results[c]["F"] for c in range(NCORES)], axis=0)
    out = _host_finish(F, w, np.asarray(b))
    return out.astype(np.float32)


# revision 31
# speedup vs baseline: 1.0002x; 1.0002x over previous
"""Trainium2 Bass kernel for nn_AdvancedRegressionModel (20-qubit circuit regression).

Math: the reference circuit collapses to out_b = sum_j g_j |(A psi_b)_j|^2 + b0
where A = kron of 20 fused 2x2 gates (RY_k @ RX_k per wire) and g absorbs the
CNOT chain (a basis permutation), the <Z_i> measurements, and the linear head
via prefix-sign weights.

Wire blocks: q = wires 0-6 (a:0-5 + m1:6), m2 = wires 7-12, z = wires 13-19.
Per batch row (4 rows/core, batch-sharded over 8 cores):
  load  [q:128, (m2:64, z:128)] fp16 in 16 contiguous chunk DMAs      (DMA)
  P0    contract q: data-stationary matmuls -> PSUM [z, (a | rh,t,h)] (PE)
  drain y [z, (rh, t, k=2*m2+h)] fp16 -- G1's columns are host-permuted
        so P2's stationary picks land CONTIGUOUS in y                 (DVE)
  P2    contract z: stationary = y[:, rh, t, :] (contiguous fp16 ->
        fast weight load), moving G2a/G2b                             (PE)
  drain l2 [k, (rh', t, z')] fp16                                     (DVE)
  P3    contract (m2,h): G3 = kron(Um2, I2) gate-stationary           (PE)
  SQ    ACT square with x512 scale -> fp16 (range-safe)               (ACT)
  P4    reduce with 16-col sign-weight stationary R -> F[10, 8192]    (PE)
  host  finish: out_r = sum_kn F[k,n] * w_k(n) / 512^2 + b

fp16 keeps 10 mantissa bits (same as f32r) -> rel err ~1.4e-3, while halving
DMA/SBUF/LDWEIGHTS cost and freeing matmuls from the f32r self-loading
single-wait constraint (no PE "touch" funnels needed).
"""

import numpy as np
from contextlib import ExitStack

import concourse.mybir as mybir
from concourse import bacc, bass, tile
from concourse.bass_utils import run_bass_kernel_spmd

NW = 20
DIM = 2 ** NW
BATCH = 32
NCORES = 8
RPC = BATCH // NCORES  # rows per core

F32 = mybir.dt.float32
F16 = mybir.dt.float16
SQ_SCALE = 512.0


# ----------------------------------------------------------------- host math
def _gates(params):
    A = []
    for k in range(NW):
        c, s = np.cos(params[k] / 2), np.sin(params[k] / 2)
        RX = np.array([[c, -1j * s], [-1j * s, c]], dtype=np.complex128)
        c2, s2 = np.cos(params[k + NW] / 2), np.sin(params[k + NW] / 2)
        RY = np.array([[c2, -s2], [s2, c2]], dtype=np.complex128)
        A.append(RY @ RX)
    return A


def _kron_list(lst):
    out = lst[0]
    for x in lst[1:]:
        out = np.kron(out, x)
    return out


def _prefix_weights(Wv, wire_lo, wire_hi):
    n = wire_hi - wire_lo + 1
    v = np.arange(2 ** n)
    signs = np.stack([1 - 2 * ((v >> (n - 1 - k)) & 1) for k in range(n)])
    pref = np.cumprod(signs, axis=0)
    g = sum(Wv[wire_lo + i] * pref[i] for i in range(n))
    return g.astype(np.float64), pref[n - 1].astype(np.float64)


def _host_precompute(params, W):
    params = np.asarray(params, dtype=np.float64)
    Wv = np.asarray(W, dtype=np.float64).reshape(-1)
    A = _gates(params)
    Gq = np.kron(_kron_list(A[0:6]), A[6])   # 128x128 on q = (a, m1)
    Uz = _kron_list(A[13:20])                # 128x128 on z
    Um2 = _kron_list(A[7:13])                # 64x64
    G3 = np.kron(Um2, np.eye(2))             # 128x128 on (m2, h)

    # G1 [q:128, nu:256], nu = rh*128 + t*2 + h with q' = h*64 + t
    G1 = np.zeros((128, 256))
    qp = np.arange(128)
    h, t = qp >> 6, qp & 63
    nu_re = t * 2 + h
    G1[:, nu_re] = Gq.real.T
    G1[:, 128 + nu_re] = Gq.imag.T

    G2a = np.concatenate([Uz.real.T, Uz.imag.T], axis=1)    # [z, 256]
    G2b = np.concatenate([-Uz.imag.T, Uz.real.T], axis=1)
    # SQ_SCALE folded into G3S so squares need no ACT scale param
    G3S = SQ_SCALE * np.concatenate(
        [G3.real.T, G3.imag.T, -G3.imag.T], axis=1)

    ga, sA = _prefix_weights(Wv, 0, 5)    # [64]
    gm, sM = _prefix_weights(Wv, 6, 12)   # [128]
    gz, _ = _prefix_weights(Wv, 13, 19)   # [128]

    # R [p3:128, 16], p3 = m2'*2 + h
    p = np.arange(128)
    m2p, hh = p >> 1, p & 1
    cols = [(hh == 0).astype(float), (hh == 1).astype(float)]
    for h0 in (0, 1):
        for m10 in (0, 1):
            cols.append((hh == h0) * gm[m10 * 64 + m2p])
    for h0 in (0, 1):
        for m10 in (0, 1):
            cols.append((hh == h0) * sM[m10 * 64 + m2p])
    R = np.stack(cols, axis=1)
    R = np.concatenate([R, np.zeros((128, 6))], axis=1)

    # free weights w[k, n], n = t*128 + z'; a' = h*32 + (t>>1), m1' = t&1
    n = np.arange(8192)
    t_, zp = n >> 7, n & 127
    m1p = t_ & 1
    w = np.zeros((10, 8192))
    w[0] = ga[0 * 32 + (t_ >> 1)]
    w[1] = ga[1 * 32 + (t_ >> 1)]
    k = 2
    for h0 in (0, 1):
        for m10 in (0, 1):
            w[k] = (m1p == m10) * sA[h0 * 32 + (t_ >> 1)]
            k += 1
    for h0 in (0, 1):
        for m10 in (0, 1):
            w[k] = (m1p == m10) * sA[h0 * 32 + (t_ >> 1)] * gz[zp]
            k += 1
    w /= SQ_SCALE ** 2

    consts = {
        "G1": np.ascontiguousarray(G1, dtype=np.float16),
        "G2a": np.ascontiguousarray(G2a, dtype=np.float16),
        "G2b": np.ascontiguousarray(G2b, dtype=np.float16),
        "G3S": np.ascontiguousarray(G3S, dtype=np.float16),
        "R": np.ascontiguousarray(R, dtype=np.float16),
    }
    return consts, w


def _host_finish(F, w, b):
    # F: [B, 10, 8192]
    out = np.einsum("bkn,kn->b", F.astype(np.float64), w)
    return out + float(np.asarray(b).reshape(-1)[0])


# ------------------------------------------------------------- bass program
def build_bass():
    nc = bacc.Bacc("TRN2", target_bir_lowering=False)
    st = nc.declare_dram_parameter("state", [RPC, DIM], F16, isOutput=False)
    g1 = nc.declare_dram_parameter("G1", [128, 256], F16, isOutput=False)
    g2a = nc.declare_dram_parameter("G2a", [128, 256], F16, isOutput=False)
    g2b = nc.declare_dram_parameter("G2b", [128, 256], F16, isOutput=False)
    g3s = nc.declare_dram_parameter("G3S", [128, 384], F16, isOutput=False)
    rw = nc.declare_dram_parameter("R", [128, 16], F16, isOutput=False)
    fout = nc.declare_dram_parameter("F", [RPC, 10, 8192], F32, isOutput=True)

    with ExitStack() as ctx:
        tc = ctx.enter_context(tile.TileContext(nc))
        const_pool = ctx.enter_context(tc.tile_pool(name="const", bufs=1))
        x0_pool = ctx.enter_context(tc.tile_pool(name="x0", bufs=2))
        y_pool = ctx.enter_context(tc.tile_pool(name="y", bufs=2))
        l2_pool = ctx.enter_context(tc.tile_pool(name="l2", bufs=4))
        sq_pool = ctx.enter_context(tc.tile_pool(name="sq", bufs=4))
        f_pool = ctx.enter_context(tc.tile_pool(name="f", bufs=4))
        psA_pool = ctx.enter_context(
            tc.tile_pool(name="psA", bufs=3, space=bass.MemorySpace.PSUM))
        psB_pool = ctx.enter_context(
            tc.tile_pool(name="psB", bufs=2, space=bass.MemorySpace.PSUM))

        G1 = const_pool.tile([128, 256], F16)
        G2a = const_pool.tile([128, 256], F16)
        G2b = const_pool.tile([128, 256], F16)
        G3S = const_pool.tile([128, 384], F16)
        R = const_pool.tile([128, 16], F16)

        def load_row(r, engines=None, split_first=0):
            # DMA dispatch serializes ~0.6us/instr per dispatching engine;
            # spreading across engines cuts time-to-first-chunk.  The first
            # `split_first` chunks load as four 128-col tiles so the first
            # P0 matmuls only wait on a 32KB transfer.
            if engines is None:
                engines = [nc.gpsimd]
            srcv = st[r].rearrange("(q f) -> q f", q=128, f=8192)
            x0s = []
            for c in range(16):
                eng = engines[c % len(engines)]
                if c < split_first:
                    parts = []
                    for j in range(4):
                        xq = x0_pool.tile([128, 128], F16, tag=f"x0{c}_{j}")
                        lo = 512 * c + 128 * j
                        eng.dma_start(xq[:], srcv[:, lo:lo + 128])
                        parts.append(xq)
                    x0s.append(parts)
                else:
                    xt = x0_pool.tile([128, 512], F16, tag=f"x0{c}")
                    eng.dma_start(xt[:], srcv[:, 512 * c:512 * c + 512])
                    x0s.append(xt)
            return x0s

        # G1 first (needed immediately), then row-0 chunks claim the DMA
        # queues, then the consts needed only from the P2 phase onward.
        nc.sync.dma_start(G1[:], g1[:])
        preload = {0: load_row(0, engines=[nc.sync, nc.gpsimd, nc.scalar])}
        nc.sync.dma_start(G2a[:], g2a[:])
        nc.sync.dma_start(G2b[:], g2b[:])
        nc.sync.dma_start(G3S[:], g3s[:])
        nc.sync.dma_start(R[:], rw[:])

        def make_y(r, x0s):
            """Emit P0 group g for row r; returns per-group closures."""
            y = y_pool.tile([128, 16384], F16, tag="y")
            y_v = y[:].rearrange("p (rh t k) -> p rh t k", rh=2, t=64, k=128)
            y_rt = y[:].rearrange("p (rt k) -> p rt k", rt=128, k=128)

            def p0_group(g):
                ps = psA_pool.tile([128, 1024], F32, tag="ps")
                for q in range(4):
                    xt = x0s[g]
                    xap = (xt[q][:] if isinstance(xt, list)
                           else xt[:, 128 * q:128 * q + 128])
                    nc.tensor.matmul(
                        ps[:, 256 * q:256 * q + 256], xap, G1[:],
                        start=True, stop=True)
                # drain: ps col = a*256 + rt*2 + h -> y col = rt*128+8g+2a+h
                pv = ps[:].rearrange("p (a rt h) -> p a rt h",
                                     a=4, rt=128, h=2)
                dst = y_rt[:, :, 8 * g:8 * g + 8].rearrange(
                    "p rt (a h) -> p a rt h", a=4, h=2)
                if g % 2 == 0:
                    nc.vector.tensor_copy(dst, pv)
                else:
                    nc.scalar.copy(dst, pv)

            return y_v, p0_group

        def stage_p2(r, y_v, g):
            ps2 = psA_pool.tile([128, 1024], F32, tag="ps")
            for q in range(4):
                t = 4 * g + q
                nc.tensor.matmul(ps2[:, 256 * q:256 * q + 256],
                                 y_v[:, 0, t, :], G2a[:],
                                 start=True, stop=False)
                nc.tensor.matmul(ps2[:, 256 * q:256 * q + 256],
                                 y_v[:, 1, t, :], G2b[:],
                                 start=False, stop=True)
            # drain: ps2 cols = (t:4, rh':2, z':128) -> l2 (rh', t, z')
            l2 = l2_pool.tile([128, 1024], F16, tag="l2")
            pv2 = ps2[:].rearrange("p (t rh z) -> p rh t z",
                                   t=4, rh=2, z=128)
            l2_v = l2[:].rearrange("p (rh t z) -> p rh t z",
                                   rh=2, t=4, z=128)
            nc.vector.tensor_copy(l2_v[:, 0], pv2[:, 0])
            nc.vector.tensor_copy(l2_v[:, 1], pv2[:, 1])
            return l2

        def stage_p3(l2):
            psR = psB_pool.tile([128, 512], F32, tag="psB")
            psI = psB_pool.tile([128, 512], F32, tag="psB")
            nc.tensor.matmul(psR[:], G3S[:, 0:128], l2[:, 0:512],
                             start=True, stop=False)
            nc.tensor.matmul(psI[:], G3S[:, 0:128],
                             l2[:, 512:1024], start=True, stop=False)
            nc.tensor.matmul(psR[:], G3S[:, 256:384],
                             l2[:, 512:1024], start=False, stop=True)
            nc.tensor.matmul(psI[:], G3S[:, 128:256],
                             l2[:, 0:512], start=False, stop=True)
            sq = sq_pool.tile([128, 1024], F16, tag="sq")
            nc.scalar.square(sq[:, 0:512], psR[:])
            nc.scalar.square(sq[:, 512:1024], psI[:])
            return sq

        def stage_p4(r, g, sq):
            ps4 = psA_pool.tile([16, 512], F32, tag="ps")
            nc.tensor.matmul(ps4[:], R[:], sq[:, 0:512],
                             start=True, stop=False)
            nc.tensor.matmul(ps4[:], R[:], sq[:, 512:1024],
                             start=False, stop=True)
            fsb = f_pool.tile([10, 512], F32, tag="fsb")
            if g % 2 == 0:
                nc.scalar.copy(fsb[:], ps4[0:10, :])
            else:
                nc.vector.tensor_copy(fsb[:], ps4[0:10, :])
            nc.sync.dma_start(fout[r][:, 512 * g:512 * g + 512], fsb[:])

        # Software-pipelined emission: P3 trails P2 by one group and P4 by
        # two, so every PE wait (l2 drained, squares done) is already
        # satisfied when PE reaches the consuming matmul.  P0 of row r+1
        # interleaves at group granularity; the shared psA pool rotation
        # cycles (ps2, ps4, ps0) kinds.
        y_v, p0g = make_y(0, preload[0])
        for g in range(16):
            p0g(g)
        for r in range(RPC):
            if r + 1 < RPC:
                x0s = load_row(r + 1)
                ny_v, np0g = make_y(r + 1, x0s)
            l2s, sqs = {}, {}
            for g in range(19):
                if g < 16:
                    l2s[g] = stage_p2(r, y_v, g)
                if 1 <= g <= 16:
                    sqs[g - 1] = stage_p3(l2s.pop(g - 1))
                if g >= 3:
                    stage_p4(r, g - 3, sqs.pop(g - 3))
                if g < 16 and r + 1 < RPC:
                    np0g(g)
            if r + 1 < RPC:
                y_v = ny_v
    nc.compile()
    return nc


# ------------------------------------------------------------------ wrapper
_CACHE = {}


def kernel(state, params, W, b):
    state = np.ascontiguousarray(np.asarray(state), dtype=np.float16)
    consts, w = _host_precompute(np.asarray(params), np.asarray(W))

    if "nc" not in _CACHE:
        _CACHE["nc"] = build_bass()
    nc = _CACHE["nc"]

    in_maps = []
    for c in range(NCORES):
        m = {"state": state[RPC * c:RPC * (c + 1)]}
        m.update(consts)
        in_maps.append(m)
    res = run_bass_kernel_spmd(nc, in_maps, list(range(NCORES)))
    F = np.concatenate([res.results[c]["F"] for c in range(NCORES)], axis=0)
    out = _host_finish(F, w, np.asarray(b))
    return out.astype(np.float32)
